# revision 1
# baseline (speedup 1.0000x reference)
"""AQT-style int8 fake-quant 3x3 conv (SAME), NHWC 32x56x56x256 -> 32x56x56x256.

Strategy (8 NeuronCores, data-parallel over batch):
  - Host: per-example quantize lhs, per-out-channel quantize rhs (exact
    integers in [-127,127] => exactly representable in bf16), pad to a
    58x58 halo and transpose to channel-major [cic,128,58*58] bf16.
  - Device (per core, 4 images): conv as 9-tap shifted matmuls on the
    TensorEngine, K = 3*3*256 contracted in 18 chunks of 128 into PSUM
    (f32, exact), dequant on VectorE with fused per-(image,channel)
    scale, DMA out channel-major f32.
  - Host: gather + transpose back to NHWC.

Raw Bass (explicit semaphores); the Tile framework's tail drain emits
multi-wait CTRL instructions this walrus build rejects.

Perf notes:
  - Each accumulation group stays on ONE PSUM bank (cycling banks per
    matmul costs ~45ns/MM in PE micro-idles; LDWEIGHTS per matmul is
    only ~4ns/MM since the PE pulls weight loads ahead).
  - dma_start costs ~0.6us serial issue time per instruction, and a
    DMA's packets are one contiguous run per partition -- so the boot
    payload (img0 rows 0-17 + all coc=0 weights) ships as TWO large-
    packet DMAs on the Sync queue; the rest of img0 goes on the Scalar
    HWDGE queue and the bulk (scales, coc=1 weights, images 1-3) on
    GpSimd SWDGE, both gated on the boot DMAs so they don't steal the
    16 shared SDMA engines from the startup-critical bytes.
  - ~100 tiny matmuls on garbage data prewarm the PE HAM clock gate
    (1.2 -> 2.4 GHz) while the boot DMAs land (~5.5us of DMA transfer
    plus completion-semaphore latency).
  - A DMA's +16 semaphore bump arrives as 16 independent per-engine
    +1s, so increments from different DMAs on one semaphore interleave:
    every wait here is for the FULL count of its semaphore (one
    semaphore per gating point), never a partial threshold.
  - No wait on the output DMAs' completion semaphore: the compiler-
    generated NEFF epilogue (exit barrier + ~7us semaphore-reset
    stream) runs after the last store's packets land.
"""

import sys

import numpy as np
import ml_dtypes

if "/opt/trn_rl_repo" not in sys.path:
    sys.path.insert(0, "/opt/trn_rl_repo")

import concourse.bass as bass
import concourse.mybir as mybir
from concourse.bass_utils import run_bass_kernel_spmd

_QMAX = 127.0

N, H, W, C = 32, 56, 56, 256
KH = KW = 3
NCORES = 8
NPER = N // NCORES          # 4 images per core
PH, PW = H + 2, W + 2       # 58x58 padded
NPAD = PH * PW              # 3364
NPIX = H * W                # 3136
RPT = 8                     # output rows per PSUM tile
NRT = H // RPT              # 7 row tiles per image
FREE = RPT * W              # 448 output pixels per matmul
NCIC = C // 128             # 2 input-channel chunks
NCOC = C // 128             # 2 output-channel chunks
NTAP = KH * KW              # 9
KSTEPS = NTAP * NCIC        # 18 matmuls per PSUM tile
TILES_PER_CORE = NPER * NCOC * NRT  # 56
NPSUM = 8                   # PSUM banks cycled
NWARM = 100                 # prewarm matmuls (N=64) to flip HAM to 2.4 GHz
# boot layout (free dim): [rows0-9 c0 | rows0-9 c1 | w coc0 c0 | w coc0 c1
#                          | rows8-17 c0 | rows8-17 c1]
# bootA = first three regions (tile 0), bootB = last two (tile 1; rows 8-9
# duplicated so each tile's window is one contiguous region)
BOOTX = 10 * PW             # 580 elements per cic per region
WOFF = NCIC * BOOTX         # weight region offset
ALEN = WOFF + NCIC * NTAP * 128      # bootA length: 3464
BOOTLEN = ALEN + NCIC * BOOTX        # 4624

I0LO = 16 * PW              # img0 rows 16-33 ride as boot DMA C
I0MID = 34 * PW             # scalar-queue img0 chunk: rows 34-57

_BF16 = mybir.dt.bfloat16
_F32 = mybir.dt.float32


def _build_nc():
    nc = bass.Bass("TRN2", num_devices=NCORES)

    boot_ext = nc.declare_dram_parameter(
        "boot", [128, BOOTLEN], _BF16, isOutput=False)
    qlhs_ext = nc.declare_dram_parameter(
        "qlhs", [NPER, NCIC, 128, NPAD], _BF16, isOutput=False)
    # coc=1 weights only (coc=0 lives in boot); free idx = (cic*NTAP+tap)*128+co
    qw1_ext = nc.declare_dram_parameter(
        "qw1", [128, NCIC * NTAP * 128], _BF16, isOutput=False)
    sc_ext = nc.declare_dram_parameter(
        "sc", [128, NCOC * NPER], _F32, isOutput=False)
    out_ext = nc.declare_dram_parameter(
        "out", [NPER, NCOC, 128, NPIX], _F32, isOutput=True)

    from contextlib import ExitStack
    with ExitStack() as ctx:
        boot_sb = ctx.enter_context(nc.sbuf_tensor("boot_sb", [128, BOOTLEN], _BF16))
        w_sb = ctx.enter_context(nc.sbuf_tensor("w_sb", [128, NCIC * NTAP * 128], _BF16))
        x_sb = [
            [ctx.enter_context(nc.sbuf_tensor(f"x_sb{i}_{c}", [128, NPAD], _BF16))
             for c in range(NCIC)]
            for i in range(NPER)
        ]
        o_sb = ctx.enter_context(
            nc.sbuf_tensor("o_sb", [128, TILES_PER_CORE * FREE], _F32))
        sc_sb = ctx.enter_context(nc.sbuf_tensor("sc_sb", [128, NCOC * NPER], _F32))
        ps = [ctx.enter_context(nc.psum_tensor(f"ps{i}", [128, FREE], _F32))
              for i in range(NPSUM)]

        # One semaphore per gating point: a DMA's +16 arrives as 16
        # per-engine +1s, so increments from different DMAs interleave --
        # waiting on a PARTIAL threshold of a shared semaphore is racy.
        bootAsem = ctx.enter_context(nc.semaphore("bootAsem"))
        bootBsem = ctx.enter_context(nc.semaphore("bootBsem"))
        bootCsem = ctx.enter_context(nc.semaphore("bootCsem"))
        i0sem = ctx.enter_context(nc.semaphore("i0sem"))
        scsem = ctx.enter_context(nc.semaphore("scsem"))
        w1sem = ctx.enter_context(nc.semaphore("w1sem"))
        qsem = [ctx.enter_context(nc.semaphore(f"qsem{i}")) for i in (1, 2, 3)]
        mmsem = ctx.enter_context(nc.semaphore("mmsem"))
        dqsem = ctx.enter_context(nc.semaphore("dqsem"))
        osem = ctx.enter_context(nc.semaphore("osem"))

        block = ctx.enter_context(nc.Block())

        LAST = TILES_PER_CORE - 1

        # tile index t decodes as (img, coc, rt), rt fastest
        def decode(t):
            img, r = divmod(t, NCOC * NRT)
            coc, rt = divmod(r, NRT)
            return img, coc, rt

        def wslice(cic, coc, tap):
            col = (cic * NTAP + tap) * 128
            if coc == 0:
                return boot_sb[:, WOFF + col: WOFF + col + 128]
            return w_sb[:, col: col + 128]

        def xview(img, cic, rt, dy, dx):
            # img0 tiles 0-1 read the boot regions (local row = dy in both)
            if img == 0 and rt == 0:
                v = (boot_sb[:, cic * BOOTX:(cic + 1) * BOOTX]
                     .rearrange("p (r c) -> p r c", c=PW))
                return v[:, dy: dy + RPT, dx: dx + W]
            if img == 0 and rt == 1:
                v = (boot_sb[:, ALEN + cic * BOOTX: ALEN + (cic + 1) * BOOTX]
                     .rearrange("p (r c) -> p r c", c=PW))
                return v[:, dy: dy + RPT, dx: dx + W]
            v = x_sb[img][cic][:].rearrange("p (r c) -> p r c", c=PW)
            r0 = rt * RPT + dy
            return v[:, r0: r0 + RPT, dx: dx + W]

        @block.sync
        def _(sync):
            sync.dma_start(boot_sb[:, :ALEN],
                           boot_ext[:, :ALEN]).then_inc(bootAsem, 16)
            sync.dma_start(boot_sb[:, ALEN:],
                           boot_ext[:, ALEN:]).then_inc(bootBsem, 16)
            # boot C: img0 rows 16-33 straight into x_sb, still on the
            # uncontended sync queue (tile 2 needs them ~7us after tile 0
            # starts; the scalar chunk behind gpsimd bulk lands too late)
            for cic in range(NCIC):
                sync.dma_start(
                    x_sb[0][cic][:, I0LO:I0MID], qlhs_ext[0, cic][:, I0LO:I0MID]
                ).then_inc(bootCsem, 16)
            for t in range(LAST):
                img, coc, rt = decode(t)
                sync.wait_ge(dqsem, t + 1)
                sync.dma_start(
                    out_ext[img, coc][:, rt * FREE:(rt + 1) * FREE],
                    o_sb[:, t * FREE:(t + 1) * FREE],
                ).then_inc(osem, 16)
            img, coc, rt = decode(LAST)
            sync.wait_ge(dqsem, LAST + 1)
            sync.dma_start(
                out_ext[img, coc][:, rt * FREE:(rt + 1) * FREE],
                o_sb[:, LAST * FREE:(LAST + 1) * FREE],
            ).then_inc(osem, 16)

        @block.scalar
        def _(scalar):
            # img0 rows 34-57 (earlier rows ride in the boot DMAs).
            # Wait for boot first: the 16 DMA engines round-robin across
            # queues, so issuing earlier would steal boot bandwidth.
            scalar.wait_ge(bootAsem, 16)
            for cic in range(NCIC):
                scalar.dma_start(
                    x_sb[0][cic][:, I0MID:], qlhs_ext[0, cic][:, I0MID:]
                ).then_inc(i0sem, 16)

        @block.gpsimd
        def _(gpsimd):
            gpsimd.wait_ge(bootAsem, 16)
            gpsimd.dma_start(sc_sb[:], sc_ext[:]).then_inc(scsem, 16)
            gpsimd.dma_start(w_sb[:], qw1_ext[:]).then_inc(w1sem, 16)
            for img in range(1, NPER):
                for cic in range(NCIC):
                    gpsimd.dma_start(
                        x_sb[img][cic][:], qlhs_ext[img, cic]
                    ).then_inc(qsem[img - 1], 16)

        @block.tensor
        def _(tensor):
            # HAM prewarm on garbage SBUF data; bank 7's first real group
            # overwrites it via start=True.
            for _ in range(NWARM):
                nc.tensor.matmul(ps[NPSUM - 1][:, :64], boot_sb[:, :128],
                                 boot_sb[:, :64], start=True, stop=True)
            tensor.wait_ge(bootAsem, 16)
            for t in range(TILES_PER_CORE):
                img, coc, rt = decode(t)
                if t == 1:
                    tensor.wait_ge(bootBsem, 16)          # img0 rows 8-17
                elif t == 2:
                    tensor.wait_ge(bootCsem, NCIC * 16)   # img0 rows 16-33
                elif t == 4:
                    tensor.wait_ge(i0sem, NCIC * 16)      # img0 rows 34-57
                elif t == NRT:
                    tensor.wait_ge(w1sem, 16)             # coc=1 weights
                elif img > 0 and coc == 0 and rt == 0:
                    tensor.wait_ge(qsem[img - 1], NCIC * 16)
                if t >= NPSUM:
                    # PSUM bank reuse: wait for dequant of tile t-NPSUM
                    tensor.wait_ge(dqsem, t - NPSUM + 1)
                mm = None
                for k in range(KSTEPS):
                    tap, cic = divmod(k, NCIC)
                    dy, dx = divmod(tap, KW)
                    mm = nc.tensor.matmul(
                        ps[t % NPSUM][:], wslice(cic, coc, tap),
                        xview(img, cic, rt, dy, dx),
                        start=(k == 0), stop=(k == KSTEPS - 1))
                mm.then_inc(mmsem, 1)

        @block.vector
        def _(vector):
            vector.wait_ge(scsem, 16)                     # scales
            for t in range(TILES_PER_CORE):
                img, coc, rt = decode(t)
                vector.wait_ge(mmsem, t + 1)
                scol = sc_sb[:, coc * NPER + img: coc * NPER + img + 1]
                nc.vector.tensor_scalar_mul(
                    o_sb[:, t * FREE:(t + 1) * FREE],
                    ps[t % NPSUM][:], scol,
                ).then_inc(dqsem, 1)

    return nc


_NC_CACHE = None


def kernel(lhs: np.ndarray, rhs: np.ndarray) -> np.ndarray:
    global _NC_CACHE
    lhs = np.asarray(lhs, dtype=np.float32)
    rhs = np.asarray(rhs, dtype=np.float32)
    assert lhs.shape == (N, H, W, C) and rhs.shape == (KH, KW, C, C)

    # --- host-side quantization (exact integers; replicated scales) ---
    amax_l = np.abs(lhs).max(axis=(1, 2, 3))                  # [N]
    s_l = np.maximum(amax_l, 1e-6) / _QMAX
    ql = np.rint(lhs / s_l[:, None, None, None]).astype(np.float32)

    amax_r = np.abs(rhs).max(axis=(0, 1, 2))                  # [C]
    s_r = np.maximum(amax_r, 1e-6) / _QMAX
    qr = np.rint(rhs / s_r[None, None, None, :]).astype(np.float32)

    # lhs -> per-core [NPER, NCIC, 128, 58*58] bf16, zero halo
    qpad = np.zeros((N, PH, PW, C), dtype=np.float32)
    qpad[:, 1:H + 1, 1:W + 1, :] = ql
    qlhs_dev = (qpad.transpose(0, 3, 1, 2)
                .reshape(N, NCIC, 128, NPAD)
                .astype(ml_dtypes.bfloat16))

    # rhs -> [NCIC, NCOC, 128, NTAP*128] bf16 (free idx = tap*128+co)
    qw_dev = (qr.reshape(NTAP, NCIC, 128, NCOC, 128)
              .transpose(1, 3, 2, 0, 4)
              .reshape(NCIC, NCOC, 128, NTAP * 128)
              .astype(ml_dtypes.bfloat16))
    # coc=1 block as [128, cic*NTAP*128]
    qw1_dev = np.ascontiguousarray(
        qw_dev[:, 1].transpose(1, 0, 2).reshape(128, NCIC * NTAP * 128))

    # fused dequant scale per (image, out-channel): sc[co128, coc*NPER+img]
    s_r2 = s_r.reshape(NCOC, 128)

    nc = _NC_CACHE
    if nc is None:
        nc = _NC_CACHE = _build_nc()

    in_maps = []
    for core in range(NCORES):
        sl = slice(core * NPER, (core + 1) * NPER)
        s_l_core = s_l[sl]
        sc = np.empty((128, NCOC * NPER), dtype=np.float32)
        for coc in range(NCOC):
            sc[:, coc * NPER:(coc + 1) * NPER] = (
                s_r2[coc][:, None] * s_l_core[None, :])
        qlhs_core = qlhs_dev[sl]
        boot = np.concatenate(
            [qlhs_core[0, 0, :, :BOOTX], qlhs_core[0, 1, :, :BOOTX],
             qw_dev[0, 0], qw_dev[1, 0],
             qlhs_core[0, 0, :, 8 * PW: 18 * PW],
             qlhs_core[0, 1, :, 8 * PW: 18 * PW]], axis=1)
        in_maps.append({
            "boot": np.ascontiguousarray(boot),
            "qlhs": qlhs_core,
            "qw1": qw1_dev,
            "sc": sc,
        })

    res = run_bass_kernel_spmd(nc, in_maps, list(range(NCORES)))

    # gather: [NPER, NCOC, 128, NPIX] f32 -> NHWC
    outs = []
    for core in range(NCORES):
        o = res.results[core]["out"]                          # [4, 2, 128, 3136]
        outs.append(o.reshape(NPER, C, NPIX).transpose(0, 2, 1)
                    .reshape(NPER, H, W, C))
    return np.concatenate(outs, axis=0).astype(np.float32)

